# revision 1
# baseline (speedup 1.0000x reference)
"""ConvKAN Trainium2 kernel (v3: 3 spline passes + short silu pass).

Decomposition (validated vs reference):
  out[(b, cin, kh, kw, q), oc] =
      sum_{func, jh, jw} Wf[oc, func, jh*48+jw] * F_func(x_pad[b, cin, 12q+jh+kh, jw+kw])
  where F_0 = silu and F_{1+g}(v) = spline cubes 4*r1^3 - r2^3 with
  t = |2.5 v + 3.5 - g|, r2 = max(2-t, 0), r1 = max(1-t, 0)
  (weights carry the -1/6 normalization).

Sharding: input channels cin split 8 ways (8 per core); the reference's
"faithful" view(-1, in_dim) maps row blocks to output channels, so core k
produces output rows [288k, 288k+288) of (B, 2304, OUT_C).

Device layout (per core):
- w_pad columns 0 and 49 always read x=0 (zero padding), so their spline
  contribution is a constant per (oc, kw) folded into a drain-time bias.
  Remaining spline slots: 8 bases x 48 w = 384 = exactly 3 passes of 128.
  Spline pass c partition p: flat = 128c + p, basis g = flat//48,
  w_pad = 1 + flat%48; per-pass x layout (host-prepped rotation).
- Silu pass: partitions (s in {0,1}) x 48 w, h-shifted by 6s (host-prepped),
  so 6 matmul steps cover jh = jj + 6s for jj in 0..5. Rows 96..127 dead.
- Per output group (kh, kw): 6 + 3*12 = 42 matmuls accumulate in PSUM;
  kw folded into zero-padded weights; kh, jh, q ride the free-dim h access
  pattern (stride 12 over 50 h rows). Silu wave runs first (its chain is one
  ACT op per chunk) so the PE starts early.
"""

from contextlib import ExitStack

import numpy as np

import concourse.bass as bass
import concourse.bacc as bacc
import concourse.tile as tile
from concourse import mybir
from concourse.alu_op_type import AluOpType
from concourse.bass_utils import run_bass_kernel_spmd

AF = mybir.ActivationFunctionType
DT = mybir.dt

B, C, H, W = 16, 64, 48, 48
OUT_C = 128
NCORES = 8
CLOC = C // NCORES          # 8 input channels per core
BC = B * CLOC               # 128 (b, c) pairs per core
HP = 50                     # padded height
FREE = HP * BC              # 6400
NSP = 3                     # spline passes
NTILE = 3 * 6 + NSP * 3 * 12  # 126 lhsT tiles: (silu kw jj) + (pass kw jh)
FCH = 4                     # free-dim chunks per activation pass
RUN_KWARGS = {}
LAST_EXEC_NS = None
N_WARMUP = 12               # HAM warm-up dummy matmuls

# engine-assignment knobs, indexed by chunk f (0..FCH-1) within each pass
S2_ON_ACT = (True, True, False, False)     # s2 square: ACT vs DVE tt
SUB_ON_GPS = (True, True, False, False)    # final sub: gpsimd vs DVE

V0 = (0.0, 0.0, -0.125, -2.875, -2.875, -0.125, 0.0, 0.0)  # slot value at x=0


def build_nc(fch: int = FCH) -> bass.Bass:
    nc = bacc.Bacc(None, target_bir_lowering=False, debug=True)
    xs = nc.declare_dram_parameter("xs", [128, FREE], DT.float16, isOutput=False)
    xp = nc.declare_dram_parameter("xp", [128, NSP * FREE], DT.float16,
                                   isOutput=False)
    wq = nc.declare_dram_parameter("wq", [128, NTILE * 128], DT.float16,
                                   isOutput=False)
    bias = nc.declare_dram_parameter("bias", [128, 8], DT.float32, isOutput=False)
    out = nc.declare_dram_parameter("out", [9, 128, 512], DT.float32, isOutput=True)

    fw = FREE // fch
    with ExitStack() as ctx:
        tc = ctx.enter_context(tile.TileContext(nc))
        wpool = ctx.enter_context(tc.tile_pool(name="w", bufs=1))
        xpool = ctx.enter_context(tc.tile_pool(name="x", bufs=2))
        fpool = ctx.enter_context(tc.tile_pool(name="f", bufs=2))
        psum_pool = ctx.enter_context(tc.tile_pool(name="ps", bufs=8, space="PSUM"))
        opool = ctx.enter_context(tc.tile_pool(name="o", bufs=4))

        bias_sb = wpool.tile([128, 8], DT.float32)
        nc.gpsimd.dma_start(bias_sb[:], bias[:])
        xs_sb = wpool.tile([128, FREE], DT.float16)
        for f in range(fch):
            sl = slice(f * fw, (f + 1) * fw)
            nc.sync.dma_start(xs_sb[:, sl], xs[:, sl])

        wq_sb = wpool.tile([128, NTILE * 128], DT.float16)
        nc.sync.dma_start(wq_sb[:, :18 * 128], wq[:, :18 * 128])  # silu tiles
        for c in range(NSP):
            wsl = slice((18 + c * 36) * 128, (18 + (c + 1) * 36) * 128)
            nc.sync.dma_start(wq_sb[:, wsl], wq[:, wsl])

        # spline-pass x layouts, double buffered; DMA kicks on gpsimd queue
        xp_sb = []
        for c in range(NSP):
            t = xpool.tile([128, FREE], DT.float16, tag="xsp")
            for f in range(fch):
                sl = slice(f * fw, (f + 1) * fw)
                nc.gpsimd.dma_start(t[:, sl], xp[:, c * FREE + f * fw:
                                                c * FREE + (f + 1) * fw])
            xp_sb.append(t)

        ts_s = wpool.tile([128, FREE], DT.float16, name="tsS", tag="tsS")
        ts_t = [wpool.tile([128, FREE], DT.float16, name=f"ts{c}", tag=f"ts{c}")
                for c in range(NSP)]

        groups = [(kh, kw) for kh in range(3) for kw in range(3)]
        ps_tiles = {}
        for g in groups[:8]:
            ps_tiles[g] = psum_pool.tile([128, 512], DT.float32,
                                         name=f"ps_{g[0]}{g[1]}", tag="ps")
        # HAM warm-up into group-7's bank (cleared by its first start=True mm)
        warm = ps_tiles[groups[7]][:]
        for _ in range(N_WARMUP):
            nc.tensor.matmul(warm, xs_sb[:, 0:128], xs_sb[:, 512:1024],
                             start=True, stop=False)

        # silu chain first: one ACT op per chunk
        for f in range(fch):
            sl = slice(f * fw, (f + 1) * fw)
            nc.scalar.activation(ts_s[:, sl], xs_sb[:, sl], AF.Silu)

        # spline chains
        for c in range(NSP):
            bias_ap = bias_sb[:, c:c + 1]
            for f in range(fch):
                sl = slice(f * fw, (f + 1) * fw)
                t = fpool.tile([128, fw], DT.float16, tag="t")
                nc.scalar.activation(t[:], xp_sb[c][:, sl], AF.Abs,
                                     bias=bias_ap, scale=2.5)
                nr2 = fpool.tile([128, fw], DT.float16, tag="nr2")  # -r2
                nc.vector.tensor_scalar(nr2[:], t[:], 2.0, 0.0,
                                        op0=AluOpType.subtract, op1=AluOpType.min)
                nr1 = fpool.tile([128, fw], DT.float16, tag="nr1")  # -r1
                nc.vector.tensor_scalar(nr1[:], t[:], 1.0, 0.0,
                                        op0=AluOpType.subtract, op1=AluOpType.min)
                s2 = fpool.tile([128, fw], DT.float16, tag="s2")    # r2^2
                if S2_ON_ACT[f]:
                    nc.scalar.activation(s2[:], nr2[:], AF.Square)
                else:
                    nc.vector.tensor_tensor(s2[:], nr2[:], nr2[:],
                                            op=AluOpType.mult)
                s1f = fpool.tile([128, fw], DT.float16, tag="s1f")  # 4 r1^2
                nc.scalar.activation(s1f[:], nr1[:], AF.Square, scale=2.0)
                c2n = fpool.tile([128, fw], DT.float16, tag="c2n")  # -r2^3
                nc.vector.tensor_tensor(c2n[:], s2[:], nr2[:], op=AluOpType.mult)
                cn1 = fpool.tile([128, fw], DT.float16, tag="cn1")  # -4 r1^3
                nc.vector.tensor_tensor(cn1[:], s1f[:], nr1[:], op=AluOpType.mult)
                eng = nc.gpsimd if SUB_ON_GPS[f] else nc.vector
                eng.tensor_tensor(ts_t[c][:, sl], c2n[:], cn1[:],
                                  op=AluOpType.subtract)

        def emit_mm(g, seq):
            kh, kw = g
            kind, c, j = seq
            if kind == "S":
                idx = kw * 6 + j
                src, first = ts_s, (j == 0)
                last = False
            else:
                idx = 18 + (c * 3 + kw) * 12 + j
                src, first = ts_t[c], False
                last = (c == NSP - 1 and j == 11)
            lhsT = wq_sb[:, idx * 128:(idx + 1) * 128]
            h0 = kh + j
            rhs = src[:].rearrange("p (h b) -> p h b", b=BC)[:, h0:h0 + 37:12, :]
            ps3 = ps_tiles[g][:].rearrange("p (q b) -> p q b", b=BC)
            nc.tensor.matmul(ps3, lhsT, rhs, start=first, stop=last)

        def drain(g):
            ob = opool.tile([128, 512], DT.float32)
            # adds the constant contribution of the removed w_pad 0/49 slots
            nc.scalar.activation(ob[:], ps_tiles[g][:], AF.Identity,
                                 bias=bias_sb[:, 4 + g[1]:5 + g[1]])
            nc.sync.dma_start(out[g[0] * 3 + g[1]], ob[:])

        wave = groups[:8]
        seqs = [("S", 0, j) for j in range(6)]
        for c in range(NSP):
            seqs += [("P", c, j) for j in range(12)]
        # waves: silu steps then spline passes for groups 0..7
        for s in seqs[:6 + 24]:
            for g in wave:
                emit_mm(g, s)
        # last spline pass: group 0 first, drain it, then the rest, then g8
        for s in seqs[30:]:
            emit_mm(wave[0], s)
        drain(wave[0])
        for g in wave[1:]:
            for s in seqs[30:]:
                emit_mm(g, s)
            drain(g)
        g8 = groups[8]
        ps_tiles[g8] = psum_pool.tile([128, 512], DT.float32, name="ps_22",
                                      tag="ps")
        for s in seqs:
            emit_mm(g8, s)
        drain(g8)
    nc.compile()
    return nc


def _prep_weights(base_weight, spline_weight, spline_scaler):
    # Wf[oc, func, jj]: func 0 = silu weights, 1+g = scaled spline / -6
    wf = np.empty((OUT_C, 9, 576), dtype=np.float64)
    wf[:, 0, :] = base_weight
    wf[:, 1:, :] = np.moveaxis(
        spline_weight.astype(np.float64)
        * spline_scaler.astype(np.float64)[..., None] / -6.0, -1, 1)
    w4 = wf.reshape(OUT_C, 9, 12, 48)
    wq = np.zeros((128, NTILE, OUT_C), dtype=np.float64)
    for kw in range(3):
        for jj in range(6):  # silu tiles
            idx = kw * 6 + jj
            for p in range(96):
                s, wp = p // 48, 1 + p % 48
                jw = wp - kw
                if 0 <= jw < 48:
                    wq[p, idx, :] = w4[:, 0, jj + 6 * s, jw]
    for c in range(NSP):
        for kw in range(3):
            for jh in range(12):
                idx = 18 + (c * 3 + kw) * 12 + jh
                for p in range(128):
                    flat = 128 * c + p
                    g, wp = flat // 48, 1 + flat % 48
                    jw = wp - kw
                    if 0 <= jw < 48:
                        wq[p, idx, :] = w4[:, 1 + g, jh, jw]
    wq = wq.reshape(128, NTILE * 128).astype(np.float16)

    bias = np.zeros((128, 8), dtype=np.float32)
    for c in range(NSP):
        for p in range(128):
            bias[p, c] = 3.5 - (128 * c + p) // 48
    # drain-time constant for removed w_pad 0 (kw=0) / 49 (kw=2) slots
    for g in range(8):
        bias[:, 4] += V0[g] * w4[:, 1 + g, :, 0].sum(axis=1)
        bias[:, 6] += V0[g] * w4[:, 1 + g, :, 47].sum(axis=1)
    return wq, bias


def _prep_x(x_slice):
    # x_slice: (B, CLOC, 48, 48) -> (xs [128, FREE], xp [128, NSP*FREE]) fp16
    plane = np.zeros((HP, HP, BC), dtype=np.float32)
    plane[1:49, 1:49, :] = np.ascontiguousarray(
        x_slice.transpose(3, 2, 0, 1)).reshape(48, 48, BC)
    flat = plane.reshape(HP, FREE)          # [w_pad, h*bc]
    sh6 = np.zeros_like(plane)              # h-shift by 6
    sh6[:, 0:44, :] = plane[:, 6:50, :]
    flat6 = sh6.reshape(HP, FREE)

    xs = np.zeros((128, FREE), dtype=np.float16)
    xs[0:48] = flat[1:49]
    xs[48:96] = flat6[1:49]
    xp = np.empty((128, NSP * FREE), dtype=np.float16)
    for c in range(NSP):
        rows = [1 + (128 * c + p) % 48 for p in range(128)]
        xp[:, c * FREE:(c + 1) * FREE] = flat[rows]
    return xs, xp


def kernel(x, base_weight, spline_weight, spline_scaler):
    x = np.asarray(x, dtype=np.float32)
    wq, bias = _prep_weights(np.asarray(base_weight), np.asarray(spline_weight),
                             np.asarray(spline_scaler))
    nc = build_nc()
    in_maps = []
    for k in range(NCORES):
        xs, xp = _prep_x(x[:, k * CLOC:(k + 1) * CLOC])
        in_maps.append({"xs": xs, "xp": xp, "wq": wq, "bias": bias})
    res = run_bass_kernel_spmd(nc, in_maps, list(range(NCORES)), **RUN_KWARGS)
    global LAST_EXEC_NS
    LAST_EXEC_NS = res.exec_time_ns
    outs = [np.asarray(r["out"]) for r in res.results]

    full = np.empty((B, 2304, OUT_C), dtype=np.float32)
    for k in range(NCORES):
        dev = outs[k].reshape(3, 3, OUT_C, 4, B, CLOC)
        rows = dev.transpose(4, 5, 0, 1, 3, 2).reshape(B, 288, OUT_C)
        full[:, 288 * k:288 * (k + 1), :] = rows
    return full.reshape(B, 128, 2304).reshape(B, 128, 48, 48)

